# revision 52
# baseline (speedup 1.0000x reference)
"""MoE (top-2 of 8 experts) Trainium2 kernel — bf16, split expert-parallel.

Strategy: the 8192 (token, expert) pairs are balanced across the 8
NeuronCores.  Plain expert-parallel puts expert e on core e and pads
every core to the busiest expert's token count; instead each expert's
token list splits into two pieces placed on an 8x2 (core, slot) grid
(a heavy expert as two slot-0 pieces, a middle one as one piece per
slot, a light one as two slot-1 pieces).  The (S1, S2) slot-capacity
frontier is searched for the smallest per-core capacity S1+S2 — PE
time scales directly with it (1037 vs 1057 for the max-count layout
on the reference routing).

The host router (softmax + top-2, renormalize) gathers and packs per
core, in bf16, every DMA descriptor contiguous and >=512B:

  xg    [128, HK, C]        the core's token stream, hk-major blocks
  w1g0/1, w2g0/1            the two experts' weights, tiled per matmul
  arms  [128, HK, cs0+128]  chunk-0 x + w1(it0) packed per hk slice
  gates [1, C]              renormalized gate per token slot (0 = pad)

On device (bf16 operands, fp32 PSUM):
  stage 1: h[it, c] = silu(w1_tile.T @ x)    (ACT Silu, I on partitions)
  stage 2: y[ht, c] = (w2_tile.T @ h) * gate (DVE mul, bf16 out)

A chunk that crosses the slot boundary splits each accumulation group
into two sub-ranges, one per weight slot.  Arms + small tensors stream
on the SP HWDGE queue while the bulk weights ride the Pool queue
(SWDGE — a second, parallel descriptor generator).  The ragged tail
chunk accumulates all its groups in a single PSUM bank → one Silu, one
gate-multiply, one compact flush on the critical tail.

The host upcasts and scatter-adds the two expert contributions per
token in fp32.
"""

import ml_dtypes
import numpy as np

import concourse.mybir as mybir
from concourse import bacc
from concourse.tile import TileContext
from concourse.bass_utils import run_bass_kernel_spmd

T, H, I, E = 4096, 1024, 1408, 8
TOPK = 2
P = 128
CHUNK = 512
HK = H // P  # 8
IT = I // P  # 11
N_CORES = 8
F32 = mybir.dt.float32
BF16 = mybir.dt.bfloat16
AF = mybir.ActivationFunctionType
BF = ml_dtypes.bfloat16

# most recently built device program (for test harnesses / cost-model timing)
LAST_NC = None


def _chunks(count):
    full, rem = divmod(count, CHUNK)
    return [CHUNK] * full + ([rem] if rem else [])


def build_moe_expert_kernel(count, s1len=None):
    """Two-half-expert MLP over a `count`-token stream; slot 0 owns token
    range [0, s1len), slot 1 owns [s1len, count).  s1len=None runs the
    whole stream against slot 0 (single-expert mode).  Pad token slots
    carry gate 0 and x 0, so they contribute nothing."""
    C = count
    nslots = 2 if s1len is not None else 1
    bnd = s1len if s1len is not None else count
    c_chunks = _chunks(count)
    c_starts = [sum(c_chunks[:j]) for j in range(len(c_chunks))]
    cs0 = c_chunks[0]
    assert bnd >= cs0, "chunk 0 must lie entirely in slot 0"
    cs_tail = c_chunks[-1]
    tail_special = (
        len(c_chunks) > 1
        and cs_tail * IT * 4 <= 2048
        and (c_starts[-1] >= bnd or bnd >= count)  # tail within one slot
    )

    def parts(ci):
        """[(chunk-relative offset, length, slot)] for chunk ci."""
        cs, c0 = c_chunks[ci], c_starts[ci]
        out = []
        if c0 < bnd:
            out.append((0, min(cs, bnd - c0), 0))
        if c0 + cs > bnd:
            off = max(0, bnd - c0)
            out.append((off, cs - off, 1))
        return out

    nc = bacc.Bacc("TRN2", target_bir_lowering=False, debug=False, num_devices=N_CORES)
    xg = nc.dram_tensor("xg", [P, HK, C], BF16, kind="ExternalInput").ap()
    w1gs, w2gs = [], []
    for s in range(nslots):
        w1gs.append(
            nc.dram_tensor(f"w1g{s}", [P, IT, HK, P], BF16, kind="ExternalInput").ap()
        )
        w2gs.append(
            nc.dram_tensor(f"w2g{s}", [P, HK, IT, P], BF16, kind="ExternalInput").ap()
        )
    gates = nc.dram_tensor("gates", [1, C], BF16, kind="ExternalInput").ap()
    # arms[:, k, :] = [x(hk_k, 0:cs0) | w1slot0(it0, hk_k)]: one small DMA
    # per hk arms both operands of stage-1 group it0's k-th matmul, so the
    # PE saturates from the very first matmul while the bulk weights stream
    arms = nc.dram_tensor("arms", [P, HK, cs0 + P], BF16, kind="ExternalInput").ap()
    yT = nc.dram_tensor("yT", [HK, P, C], BF16, kind="ExternalOutput").ap()
    if tail_special:
        # tail gate vector replicated HK times (one broadcast DMA); the
        # tail's output leaves in ONE combined flush with chunk n-2's last
        # h-row so a single DMA latency sits on the program tail
        gtail = nc.dram_tensor(
            "gtail", [1, HK * cs_tail], BF16, kind="ExternalInput"
        ).ap()
        # combined late flush: [chunk n-2's ht7 row | the whole tail]
        late_w = (CHUNK if len(c_chunks) > 1 else 0) + HK * cs_tail
        yLate = nc.dram_tensor(
            "yLate", [P, late_w], BF16, kind="ExternalOutput"
        ).ap()

    with TileContext(nc) as tc:
        with (
            tc.tile_pool(name="wpool", bufs=1) as wpool,
            tc.tile_pool(name="xpool", bufs=3) as xpool,
            tc.tile_pool(name="hpool", bufs=3) as hpool,
            tc.tile_pool(name="ypool", bufs=2) as ypool,
            tc.tile_pool(name="ps1", bufs=4, space="PSUM") as ps1pool,
            tc.tile_pool(name="ps2", bufs=4, space="PSUM") as ps2pool,
        ):
            ar = wpool.tile([P, HK, cs0 + P], BF16)
            w1ss = [
                wpool.tile([P, IT, HK, P], BF16, name=f"w1s{s}") for s in range(nslots)
            ]
            w2ss = [
                wpool.tile([P, HK, IT, P], BF16, name=f"w2s{s}") for s in range(nslots)
            ]
            gb = wpool.tile([P, C], BF16)

            xs_tiles = {}

            def load_x(ci):
                xs = xpool.tile([P, HK, CHUNK], BF16, tag="xs", name=f"xs{ci}")
                cs, c0 = c_chunks[ci], c_starts[ci]
                nc.gpsimd.dma_start(xs[:, :, :cs], xg[:, :, c0 : c0 + cs])
                xs_tiles[ci] = xs

            # -- input streams, in consumption order ----------------------
            # Two parallel DMA-issue paths: the SP queue (HWDGE) carries the
            # arms + small tensors, while the Pool queue (SWDGE — its own
            # descriptor generator, bypasses the serial HWDGE) streams the
            # bulk w1/w2 weights and the x chunks concurrently.
            for hk in range(HK):
                nc.sync.dma_start(ar[:, hk, :], arms[:, hk, :])
            for it in range(1, IT):
                nc.gpsimd.dma_start(w1ss[0][:, it], w1gs[0][:, it])
            if nslots > 1:
                for it in range(0, 3):
                    nc.gpsimd.dma_start(w1ss[1][:, it], w1gs[1][:, it])
            for ci in range(1, len(c_chunks)):
                load_x(ci)
            if nslots > 1:
                for it in range(3, IT):
                    nc.gpsimd.dma_start(w1ss[1][:, it], w1gs[1][:, it])
            # gates ride the Pool queue late: on the SP queue they hit
            # the bus right after the arms and delay the hot w1 stream
            nc.gpsimd.dma_start(gb[:], gates[0].partition_broadcast(P))
            if tail_special:
                gbt = wpool.tile([P, HK * cs_tail], BF16)
                nc.gpsimd.dma_start(gbt[:], gtail[0].partition_broadcast(P))
            for s in range(nslots):
                for b in range(0, HK, 2):
                    nc.gpsimd.dma_start(w2ss[s][:, b : b + 2], w2gs[s][:, b : b + 2])

            hs_tiles = {}

            def _lhs1(it, hk, slot=0):
                if slot == 0 and it == 0:
                    return ar[:, hk, cs0 : cs0 + P]
                return w1ss[slot][:, it, hk, :]

            def _rhs1(ci, hk, lo, ln):
                if ci == 0:
                    return ar[:, hk, lo : lo + ln]
                return xs_tiles[ci][:, hk, lo : lo + ln]

            def stage1(ci, wavefront=False):
                cs = c_chunks[ci]
                cparts = parts(ci)
                hs = hpool.tile([P, IT, CHUNK], BF16, tag="hs", name=f"hs{ci}")

                def groups(it, ps1):
                    for lo, ln, slot in cparts:
                        for hk in range(HK):
                            nc.tensor.matmul(
                                ps1[:, lo : lo + ln],
                                _lhs1(it, hk, slot),
                                _rhs1(ci, hk, lo, ln),
                                start=(hk == 0),
                                stop=(hk == HK - 1),
                            )

                def evac(it, ps1):
                    nc.scalar.activation(hs[:, it, :cs], ps1[:, :cs], AF.Silu)

                if wavefront:
                    # chunk 0: x hk-slices (arms) and w1 i-tiles are still
                    # streaming in.  Emit the first 4 accumulation groups in
                    # predicted-arrival order (arms land ~1.2us apart on the
                    # bus, w1 i-tiles ~1.6us apart) so the in-order PE queue
                    # never stalls on a matmul whose data trails another's.
                    front = min(4, IT)
                    pss = {
                        it: ps1pool.tile([P, CHUNK], F32, tag="ps1", name=f"ps1wf{it}")
                        for it in range(front)
                    }
                    arm_vis = [3321, 4504, 4959, 6142, 7325, 7780, 8963, 9418]
                    w1_vis = [0, 4049, 5687, 6870]
                    est = sorted(
                        ((max(arm_vis[hk % 8], w1_vis[min(it, 3)]), hk, it)
                         for hk in range(HK) for it in range(front)),
                    )
                    # within a group, hk order must stay ascending for the
                    # start/stop accumulation flags; sorting by arrival keeps
                    # it ascending per it (both estimates grow with hk)
                    for _, hk, it in est:
                        nc.tensor.matmul(
                            pss[it][:, :cs],
                            _lhs1(it, hk),
                            _rhs1(ci, hk, 0, cs),
                            start=(hk == 0),
                            stop=(hk == HK - 1),
                        )
                    for it in range(front):
                        evac(it, pss[it])
                    rest = range(front, IT)
                else:
                    rest = range(IT)
                for it in rest:
                    ps1 = ps1pool.tile([P, CHUNK], F32, tag="ps1")
                    groups(it, ps1)
                    evac(it, ps1)
                hs_tiles[ci] = hs

            def stage1_tail(ci):
                # all IT accumulation groups land in ONE PSUM bank, then a
                # single Silu evacuates the whole chunk: no ps1 recycling
                # and no per-group ACT pacing on the program's critical tail
                cs = c_chunks[ci]
                cparts = parts(ci)
                pad = 2048 // (4 * IT)
                ps1t = ps1pool.tile([P, IT, pad], F32, tag="ps1", name="ps1tail")
                for it in range(IT):
                    for lo, ln, slot in cparts:
                        for hk in range(HK):
                            nc.tensor.matmul(
                                ps1t[:, it, lo : lo + ln],
                                _lhs1(it, hk, slot),
                                _rhs1(ci, hk, lo, ln),
                                start=(hk == 0),
                                stop=(hk == HK - 1),
                            )
                hst = hpool.tile([P, IT, cs], BF16, tag="hs", name="hstail")
                nc.scalar.activation(hst[:, :, :], ps1t[:, :, :cs], AF.Silu)
                hs_tiles[ci] = hst

            def stage2(ci, last=False, late_tile=None):
                cs, c0 = c_chunks[ci], c_starts[ci]
                cparts = parts(ci)
                hs = hs_tiles.pop(ci)
                ys = ypool.tile([P, HK, CHUNK], BF16, tag="ys")
                # big chunks flush in two DMAs so the first part's transfer
                # overlaps the rest of the chunk's matmuls; flushes ride the
                # (idle after the input prologue) SP queue.  When a late
                # tile is given, ht7's output joins the tail's combined
                # flush instead of paying its own DMA on the critical tail.
                flush_at = [HK - 1] if last else [6, HK - 1]
                if late_tile is not None:
                    flush_at = [6]
                h0 = 0
                for ht in range(HK):
                    ps2 = ps2pool.tile([P, CHUNK], F32, tag="ps2")
                    for lo, ln, slot in cparts:
                        for it in range(IT):
                            nc.tensor.matmul(
                                ps2[:, lo : lo + ln],
                                w2ss[slot][:, ht, it, :],
                                hs[:, it, lo : lo + ln],
                                start=(it == 0),
                                stop=(it == IT - 1),
                            )
                    if late_tile is not None and ht == HK - 1:
                        mul_out = late_tile[:, :CHUNK][:, :cs]
                    else:
                        mul_out = ys[:, ht, :cs]
                    nc.vector.tensor_mul(
                        out=mul_out, in0=ps2[:, :cs], in1=gb[:, c0 : c0 + cs]
                    )
                    if ht in flush_at:
                        # DRAM AP reordered to partition-major to match SBUF
                        out_ap = yT[h0 : ht + 1, :, c0 : c0 + cs].rearrange(
                            "ht p c -> p ht c"
                        )
                        nc.sync.dma_start(out_ap, ys[:, h0 : ht + 1, :cs])
                        h0 = ht + 1

            def stage2_tail(ci, late_tile):
                # same single-bank trick for stage 2: one DVE gate-multiply
                # over all HK groups; the tail's output shares one combined
                # flush with chunk n-2's ht7 row on the idle SP queue
                cs = c_chunks[ci]
                cparts = parts(ci)
                hst = hs_tiles.pop(ci)
                pad = 2048 // (4 * HK)
                ps2t = ps2pool.tile([P, HK, pad], F32, tag="ps2", name="ps2tail")
                for ht in range(HK):
                    for lo, ln, slot in cparts:
                        for it in range(IT):
                            nc.tensor.matmul(
                                ps2t[:, ht, lo : lo + ln],
                                w2ss[slot][:, ht, it, :],
                                hst[:, it, lo : lo + ln],
                                start=(it == 0),
                                stop=(it == IT - 1),
                            )
                base = late_tile.shape[1] - HK * cs
                nc.vector.tensor_mul(
                    out=late_tile[:, base:].rearrange("p (ht c) -> p ht c", ht=HK),
                    in0=ps2t[:, :, :cs],
                    in1=gbt[:].rearrange("p (ht c) -> p ht c", ht=HK),
                )
                nc.sync.dma_start(yLate[:], late_tile[:])

            # software pipeline: stage 1 runs a chunk ahead so the PE has
            # work while w2 is still streaming in.  The ragged tail's
            # stage 1 runs early (its Silu latency hides under big-chunk
            # stage-2 work); only tail stage 2 -> one mul -> one flush sit
            # on the program's critical tail.
            n = len(c_chunks)
            stage1(0, wavefront=True)
            if n > 1 and tail_special:
                late = ypool.tile([P, CHUNK + HK * cs_tail], BF16, name="late")
                if n > 2:
                    stage1(1)
                stage1_tail(n - 1)
                stage2(0, late_tile=late if n == 2 else None)
                for ci in range(2, n - 1):
                    stage1(ci)
                    stage2(ci - 1)
                if n > 2:
                    stage2(n - 2, late_tile=late)
                stage2_tail(n - 1, late)
            elif n > 1:
                stage1(1)
                stage2(0)
                for ci in range(2, n):
                    stage1(ci)
                    stage2(ci - 1)
                stage2(n - 1, last=True)
            else:
                stage2(0, last=True)
    nc.compile()
    global LAST_NC
    LAST_NC = nc
    return nc


def route(router_logits):
    """Host-side router: softmax -> top-2 -> renormalize.

    Returns (top2_idx [T,2] int64, top2_gate [T,2] float32)."""
    logits = np.asarray(router_logits, dtype=np.float32)
    m = logits.max(axis=-1, keepdims=True)
    ex = np.exp(logits - m)
    probs = ex / ex.sum(axis=-1, keepdims=True)
    order = np.argsort(-probs, axis=-1, kind="stable")[:, :TOPK]
    rows = np.arange(logits.shape[0])[:, None]
    topk_p = probs[rows, order]
    topk_p = topk_p / topk_p.sum(axis=-1, keepdims=True)
    return order, topk_p.astype(np.float32)


def _pack_w1(w1e):
    # w1g[p, it, hk, j] = w1e[it*128+j, hk*128+p]
    return np.ascontiguousarray(
        w1e.reshape(IT, P, HK, P).transpose(3, 0, 2, 1)
    ).astype(BF)


def _pack_w2(w2e):
    # w2g[p, ht, it, j] = w2e[ht*128+j, it*128+p]
    return np.ascontiguousarray(
        w2e.reshape(HK, P, IT, P).transpose(3, 0, 2, 1)
    ).astype(BF)


def _pack_x(xf):
    # xg[p, hk, c] = xf[c, hk*128+p]
    count = xf.shape[0]
    return np.ascontiguousarray(xf.T.reshape(HK, P, count).transpose(1, 0, 2)).astype(
        BF
    )


def kernel(x, router_logits, w1, w2):
    x = np.ascontiguousarray(np.asarray(x, dtype=np.float32))
    w1 = np.asarray(w1, dtype=np.float32)
    w2 = np.asarray(w2, dtype=np.float32)
    t = x.shape[0]

    top2_idx, top2_gate = route(router_logits)

    expert_tokens = []
    expert_gates = []
    for e in range(E):
        sel = np.nonzero(top2_idx == e)
        expert_tokens.append(sel[0])
        expert_gates.append(top2_gate[sel[0], sel[1]])
    counts = np.array([len(ix) for ix in expert_tokens])

    # split mode: every expert's token list splits into two pieces placed in
    # the 8x2 (core, slot) grid; per-core capacity = S1 + S2 where S1/S2
    # bound the slot-0/slot-1 piece sizes.  An expert may sit as two slot-0
    # pieces (heavy), one piece per slot (middle), or two slot-1 pieces
    # (light) — search the (S1, S2) frontier for the smallest capacity.
    def _assign(S1, S2):
        order = np.argsort(-counts, kind="stable")
        heavy = [int(e) for e in order if counts[e] > S1 + S2]
        a = len(heavy)
        if a > 4 or (heavy and int(counts[heavy[0]]) > 2 * S1):
            return None
        light = [int(e) for e in order[::-1][:a]]
        if light and int(counts[light[-1]]) > 2 * S2:
            return None
        mid = [int(e) for e in order if e not in heavy and e not in light]
        cells0, cells1 = [], []  # pieces: (expert, start, length)
        for e in heavy:
            h = (int(counts[e]) + 1) // 2
            cells0 += [(e, 0, h), (e, h, int(counts[e]) - h)]
        for e in light:
            h = (int(counts[e]) + 1) // 2
            cells1 += [(e, 0, h), (e, h, int(counts[e]) - h)]
        for e in mid:
            s = max(0, int(counts[e]) - S2)
            cells0.append((e, 0, s))
            cells1.append((e, s, int(counts[e]) - s))
        if len(cells0) != N_CORES or len(cells1) != N_CORES:
            return None
        return [[cells0[k], cells1[k]] for k in range(N_CORES)]

    cmax = int(counts.max())
    best = None
    for S1 in range(max(CHUNK, -(-cmax // 2)), cmax + 1):
        # smallest S2 for this S1 by direct scan
        lo, hi = 0, cmax
        while lo < hi:
            midv = (lo + hi) // 2
            if _assign(S1, midv) is not None:
                hi = midv
            else:
                lo = midv + 1
        if _assign(S1, lo) is not None and (best is None or S1 + lo < best[0] + best[1]):
            best = (S1, lo)
    split_ok = (
        E == 8 and N_CORES == 8 and best is not None and sum(best) < max(CHUNK, cmax)
    )

    if split_ok:
        S1, S2 = best
        count, s1len = S1 + S2, S1
        grid = _assign(S1, S2)
        pieces = [
            [(e, slice(st, st + ln)) for e, st, ln in core] for core in grid
        ]
    else:
        count, s1len = max(CHUNK, cmax), None
        pieces = [[(e, slice(0, None))] for e in range(E)]

    chunks = _chunks(count)
    cs0 = chunks[0]
    cs_tail = chunks[-1]
    tail_c0 = count - cs_tail
    tail_special = (
        len(chunks) > 1
        and cs_tail * IT * 4 <= 2048
        and (tail_c0 >= (s1len or count) or (s1len or count) >= count)
    )

    nc = build_moe_expert_kernel(count, s1len)

    w1p = [_pack_w1(w1[e]) for e in range(E)]
    w2p = [_pack_w2(w2[e]) for e in range(E)]

    in_maps = []
    core_slots = []
    for core in range(N_CORES):
        segs = pieces[core]
        bounds = [0, s1len] if s1len is not None else [0]
        xf = np.zeros((count, H), dtype=np.float32)
        g = np.zeros((1, count), dtype=np.float32)
        slots = []
        im = {}
        for si, (e, sl) in enumerate(segs):
            rows = expert_tokens[e][sl]
            base = bounds[si]
            xf[base : base + len(rows)] = x[rows]
            g[0, base : base + len(rows)] = expert_gates[e][sl]
            im[f"w1g{si}"] = w1p[e]
            im[f"w2g{si}"] = w2p[e]
            slots.append((e, rows, base))
        xgc = _pack_x(xf)
        im["xg"] = xgc
        im["gates"] = g.astype(BF)
        im["arms"] = np.ascontiguousarray(
            np.concatenate([xgc[:, :, :cs0], im["w1g0"][:, 0, :, :]], axis=2)
        )
        if tail_special:
            im["gtail"] = np.tile(g[:, tail_c0:].astype(BF), (1, HK))
        in_maps.append(im)
        core_slots.append(slots)

    def run_and_assemble():
        res = run_bass_kernel_spmd(nc, in_maps, core_ids=list(range(N_CORES)))
        out = np.zeros((t, H), dtype=np.float32)
        for core in range(N_CORES):
            yT = np.asarray(res.results[core]["yT"], dtype=np.float32).reshape(
                H, count
            )
            if tail_special:
                # chunk n-2's ht7 row and the whole tail arrive via the
                # combined late-flush tensor
                late = np.asarray(res.results[core]["yLate"], dtype=np.float32)
                c1_start = count - cs_tail - CHUNK
                yT[(HK - 1) * P :, c1_start : c1_start + CHUNK] = late[:, :CHUNK]
                yT[:, tail_c0:] = (
                    late[:, CHUNK:]
                    .reshape(P, HK, cs_tail)
                    .transpose(1, 0, 2)
                    .reshape(H, cs_tail)
                )
            for e, rows, base in core_slots[core]:
                out[rows] += yT[:, base : base + len(rows)].T
        return out

    out = run_and_assemble()
    if not np.isfinite(out).all():
        # one retry in case of a transient device fault
        out = run_and_assemble()
    return out
